# revision 2
# baseline (speedup 1.0000x reference)
"""Trainium2 Bass kernel for nn_BAGCA, v2.

Data-parallel over batch (32 -> 8 cores x 4). Natural-layout scores +
exp once; transposed attention matrix via DMA xbar transposes; softmax
normalization folded into replicated-rowsum reciprocal (ctx_d),
rinv-scaled Vd (ctx_p) and rinv-weighted colsums (fc_pd). Sigmoids
computed as 0.5*(1+tanh(x/2)) so ACT stays on the exp table set.
bf16 projection path throughout.
"""
import math
import numpy as np

import concourse.bacc as bacc
import concourse.mybir as mybir
import concourse.tile as tile
from concourse.bass_utils import run_bass_kernel_spmd

F32 = mybir.dt.float32
F32R = mybir.dt.float32r
BF16 = mybir.dt.bfloat16
AF = mybir.ActivationFunctionType
ALU = mybir.AluOpType

H = 256
B = 32
DL = 290
PL = 1000
PLP = 1024
NCORES = 8
BPC = B // NCORES
DIC = [128, 128, 34]
DOF = [0, 128, 256]
SOF = [0, 128, 256]
SBLK = [128, 128, 48]
NST = 304
NJC = 8
PJN = [128] * 7 + [104]

_CACHE = {}


def _pe_table(length, d):
    pos = np.arange(length, dtype=np.float32)[:, None]
    div = np.exp(np.arange(0, d, 2, dtype=np.float32)
                 * np.float32(-math.log(10000.0) / d))
    ang = pos * div
    pe = np.zeros((length, d), np.float32)
    pe[:, 0::2] = np.sin(ang)
    pe[:, 1::2] = np.cos(ang)
    return pe


def _sigmoid(x):
    return 1.0 / (1.0 + np.exp(-x))


def _host_prep(inp):
    a = float(_sigmoid(inp["alpha"]))
    isq = 1.0 / math.sqrt(32)
    wqd = inp["w_q_d"] * np.float32(a * isq)
    bqd = inp["b_q_d"] * np.float32(a * isq)
    wqp = inp["w_q_p"] * np.float32((1.0 - a) * isq)
    bqp = inp["b_q_p"] * np.float32((1.0 - a) * isq)

    drug_attn = (1.0 / PL) * inp["w_fc_dp"].sum(0) + inp["b_fc_dp"]
    sig_dp = _sigmoid(drug_attn).astype(np.float32)
    woutd = inp["w_out_d"] * sig_dp[None, :]
    boutd = inp["b_out_d"] * sig_dp

    ws = [wqd, inp["w_k_d"], inp["w_k_p"], wqp, inp["w_v_d"], inp["w_v_p"],
          inp["w_gd"], inp["w_gp"], woutd, inp["w_out_p"]]
    wt = np.stack([w.reshape(2, 128, H) for w in ws]).astype(np.float32)

    tbs = [bqd, inp["b_k_d"], inp["b_k_p"], bqp]
    tb = np.stack([b.reshape(2, 128, 1) for b in tbs]).astype(np.float32)

    nb = np.stack([inp["b_v_d"], inp["b_v_p"], inp["b_gd"], inp["b_gp"],
                   boutd, inp["b_out_p"], inp["b_fc_pd"]])[None].astype(
                       np.float32)

    wfcg = np.zeros((2, 128, H), np.float32)
    for g in range(2):
        for h4 in range(4):
            wfcg[g, 32 * h4, :] = inp["w_fc_pd"][4 * g + h4] / DL

    ped = (inp["scale_d"].reshape(-1)[0] * _pe_table(DL, H)).astype(np.float32)
    pep = (inp["scale_p"].reshape(-1)[0] * _pe_table(PL, H)).astype(np.float32)

    ident = np.eye(128, dtype=np.float32)
    onesrow = np.ones((1, 512), np.float32)
    return dict(wt=wt, tb=tb, nb=nb, wfcg=wfcg, ped=ped, pep=pep,
                ident=ident, onesrow=onesrow)


def _build():
    nc = bacc.Bacc("TRN2", target_bir_lowering=False, debug=False,
                   num_devices=NCORES)
    drug = nc.dram_tensor("drug", [BPC, DL, H], F32, kind="ExternalInput")
    prot = nc.dram_tensor("prot", [BPC, PL, H], F32, kind="ExternalInput")
    wt_d = nc.dram_tensor("wt", [10, 2, 128, H], F32, kind="ExternalInput")
    tb_d = nc.dram_tensor("tb", [4, 2, 128, 1], F32, kind="ExternalInput")
    nb_d = nc.dram_tensor("nb", [1, 7, H], F32, kind="ExternalInput")
    wfcg_d = nc.dram_tensor("wfcg", [2, 128, H], F32, kind="ExternalInput")
    ped_d = nc.dram_tensor("ped", [DL, H], F32, kind="ExternalInput")
    pep_d = nc.dram_tensor("pep", [PL, H], F32, kind="ExternalInput")
    id_d = nc.dram_tensor("ident", [128, 128], F32, kind="ExternalInput")
    or_d = nc.dram_tensor("onesrow", [1, 512], F32, kind="ExternalInput")
    out = nc.dram_tensor("out", [BPC, DL + PL, H], F32, kind="ExternalOutput")

    from contextlib import ExitStack
    with tile.TileContext(nc) as tc, ExitStack() as stack:
        cp = stack.enter_context(tc.tile_pool(name="const", bufs=1))
        io = stack.enter_context(tc.tile_pool(name="io", bufs=2))
        at = stack.enter_context(tc.tile_pool(name="attn", bufs=1))
        ps = stack.enter_context(tc.tile_pool(name="ps", bufs=1, space="PSUM"))

        # ---- constants ----
        stg = cp.tile([128, 2, H], F32, tag="stg")
        wsb = []          # QK/V/G weights in bf16; OD f32r; OP bf16
        for w in range(8):
            t = cp.tile([128, 2, H], BF16, tag=f"w{w}", name=f"w{w}")
            nc.sync.dma_start(stg[:], wt_d[w])
            nc.vector.tensor_copy(t[:], stg[:])
            wsb.append(t)
        wod = cp.tile([128, 2, H], F32R, tag="wod")
        nc.sync.dma_start(stg[:], wt_d[8])
        nc.vector.tensor_copy(wod[:], stg[:])
        wsb.append(wod)
        wopb = cp.tile([128, 2, H], BF16, tag="wopb")
        nc.sync.dma_start(stg[:], wt_d[9])
        nc.vector.tensor_copy(wopb[:], stg[:])
        wsb.append(wopb)
        wfcb = []
        for g in range(2):
            t = cp.tile([128, H], BF16, tag=f"wfcb{g}", name=f"wfcb{g}")
            nc.sync.dma_start(stg[:, 0], wfcg_d[g])
            nc.vector.tensor_copy(t[:], stg[:, 0])
            wfcb.append(t)
        tbs = []
        for p in range(4):
            t = cp.tile([128, 2, 1], F32, tag=f"tb{p}", name=f"tb{p}")
            nc.sync.dma_start(t[:], tb_d[p])
            tbs.append(t)
        nbf = cp.tile([1, 7, H], F32, tag="nbf")
        nc.sync.dma_start(nbf[:], nb_d[:])
        nbb = cp.tile([1, 7, H], BF16, tag="nbb")
        nc.vector.tensor_copy(nbb[:], nbf[:])
        nbrod = cp.tile([1, 1, H], F32R, tag="nbrod")
        nc.vector.tensor_copy(nbrod[:], nbf[:, 4:5])
        idf = cp.tile([128, 128], F32, tag="idf")
        nc.sync.dma_start(idf[:], id_d[:])
        idr = cp.tile([128, 128], F32R, tag="idr")
        nc.vector.tensor_copy(idr[:], idf[:])
        idb = cp.tile([128, 128], BF16, tag="idb")
        nc.vector.tensor_copy(idb[:], idf[:])
        onef = cp.tile([1, 512], F32, tag="onef")
        nc.sync.dma_start(onef[:], or_d[:])
        onerr = cp.tile([1, 512], F32R, tag="onerr")
        nc.vector.tensor_copy(onerr[:], onef[:])
        onesb = cp.tile([128, 32], BF16, tag="onesb")
        nc.vector.memset(onesb[:], 1.0)
        onerb = cp.tile([1, 512], BF16, tag="onerb")
        nc.vector.memset(onerb[:], 1.0)
        ped_t = cp.tile([128, 3, H], BF16, tag="ped_t")
        nc.sync.dma_start(stg[:, 0:2, :],
                          ped_d[0:256].rearrange("(c p) h -> p c h", p=128))
        nc.vector.tensor_copy(ped_t[:, 0:2], stg[:])
        nc.sync.dma_start(stg[:34, 0, :], ped_d[256:290])
        nc.vector.tensor_copy(ped_t[:34, 2], stg[:34, 0])
        pep_t = cp.tile([128, 8, H], BF16, tag="pep_t")
        for k in range(3):
            nc.sync.dma_start(
                stg[:, 0:2, :],
                pep_d[256 * k:256 * (k + 1)].rearrange("(c p) h -> p c h",
                                                       p=128))
            nc.vector.tensor_copy(pep_t[:, 2 * k:2 * k + 2], stg[:])
        nc.sync.dma_start(stg[:, 0, :], pep_d[768:896])
        nc.vector.tensor_copy(pep_t[:, 6], stg[:, 0])
        nc.sync.dma_start(stg[:104, 0, :], pep_d[896:1000])
        nc.vector.tensor_copy(pep_t[:104, 7], stg[:104, 0])

        for b in range(BPC):
            _bel(nc, io, at, ps, b, drug, prot, out, wsb, wfcb, tbs, nbb,
                 nbrod, idr, idb, onerr, onesb, onerb, ped_t, pep_t)

    nc.compile()
    return nc


def _bel(nc, io, at, ps, b, drug, prot, out, wsb, wfcb, tbs, nbb,
         nbrod, idr, idb, onerr, onesb, onerb, ped_t, pep_t):
    QD, KD, KP, QP, VD, VP, GD, GP, OD, OP = range(10)

    def pj():
        return ps.tile([128, 512], F32, tag="pj", name="pj", bufs=2)

    def ptp():
        return ps.tile([128, 512], BF16, tag="ptb", name="ptb", bufs=1)

    def psc():
        return ps.tile([128, 2, 512], F32, tag="psc", name="psc", bufs=2)

    def pcx():
        return ps.tile([128, 512], F32, tag="pcx", name="pcx", bufs=1)

    # ---- load inputs (staged) + positional encoding -> bf16 Xe ----
    De = io.tile([128, 3, H], BF16, tag="De")
    Pe = io.tile([128, 8, H], BF16, tag="Pe")
    ldst = io.tile([128, 2, H], F32, tag="ldst")
    nc.gpsimd.dma_start(ldst[:],
                      drug[b, 0:256].rearrange("(c p) h -> p c h", p=128))
    for ic in range(2):
        nc.vector.tensor_add(De[:, ic], ldst[:, ic], ped_t[:, ic])
    ldst2 = io.tile([128, 2, H], F32, tag="ldst2")
    nc.gpsimd.dma_start(ldst2[:34, 0, :], drug[b, 256:290])
    nc.vector.tensor_add(De[:34, 2], ldst2[:34, 0], ped_t[:34, 2])
    for k in range(3):
        st = io.tile([128, 2, H], F32, tag="ldst" if k % 2 == 0 else "ldst2",
                     name="pst")
        nc.gpsimd.dma_start(
            st[:], prot[b, 256 * k:256 * (k + 1)].rearrange(
                "(c p) h -> p c h", p=128))
        for q in range(2):
            nc.vector.tensor_add(Pe[:, 2 * k + q], st[:, q],
                                 pep_t[:, 2 * k + q])
    st = io.tile([128, 2, H], F32, tag="ldst2", name="pst2")
    nc.gpsimd.dma_start(st[:, 0, :], prot[b, 768:896])
    nc.gpsimd.dma_start(st[:104, 1, :], prot[b, 896:1000])
    nc.vector.tensor_add(Pe[:, 6], st[:, 0], pep_t[:, 6])
    nc.vector.tensor_add(Pe[:104, 7], st[:104, 1], pep_t[:104, 7])

    # ---- transpose inputs (bf16) ----
    DeT = at.tile([128, 2, DL], BF16, tag="DeT")
    for chb in range(2):
        p = ptp()
        for ic in range(3):
            nc.tensor.transpose(p[:, DOF[ic]:DOF[ic] + DIC[ic]],
                                De[:DIC[ic], ic, 128 * chb:128 * chb + 128],
                                idb[:DIC[ic], :DIC[ic]])
        nc.vector.tensor_copy(DeT[:, chb, :], p[:, :DL])
    PeT = at.tile([128, 2, PLP], BF16, tag="PeT")
    for chb in range(2):
        for hf in range(2):
            p = ptp()
            for k in range(4):
                jc = 4 * hf + k
                nc.tensor.transpose(
                    p[:, 128 * k:128 * k + PJN[jc]],
                    Pe[:PJN[jc], jc, 128 * chb:128 * chb + 128],
                    idb[:PJN[jc], :PJN[jc]])
            nc.vector.tensor_copy(PeT[:, chb, hf * 512:hf * 512 + 512],
                                  p[:, :512])

    # ---- transposed Q/K projections -> bf16 ----
    QdT = at.tile([128, 2, DL], BF16, tag="QdT")
    KdT = at.tile([128, 2, DL], BF16, tag="KdT")
    KpT = at.tile([128, 2, PLP], BF16, tag="KpT")
    QpT = at.tile([128, 2, PLP], BF16, tag="QpT")
    specs = [(QD, DeT, [(0, DL)], QdT, 0), (KD, DeT, [(0, DL)], KdT, 1),
             (KP, PeT, [(0, 512), (512, 512)], KpT, 2),
             (QP, PeT, [(0, 512), (512, 512)], QpT, 3)]
    for w, xT, halves, outT, tbi in specs:
        for mc in range(2):
            for off, ln in halves:
                p = pj()
                for kc in range(2):
                    nc.tensor.matmul(
                        p[:, :ln], wsb[w][:, kc, 128 * mc:128 * mc + 128],
                        xT[:, kc, off:off + ln],
                        start=(kc == 0), stop=(kc == 1))
                nc.vector.tensor_scalar_add(outT[:, mc, off:off + ln],
                                            p[:, :ln], tbs[tbi][:, mc])
    nc.vector.memset(KpT[:, :, PL:PLP], 0.0)
    nc.vector.memset(QpT[:, :, PL:PLP], 0.0)

    # ---- V / gate projections (natural, bf16); gates as sigmoids ----
    Vd = io.tile([128, 3, H], BF16, tag="Vd")
    gd = io.tile([128, 3, H], BF16, tag="gd")
    for ic in range(3):
        n = DIC[ic]
        pv = pj()
        for kc in range(2):
            nc.tensor.matmul(pv[:n, :H], DeT[:, kc, DOF[ic]:DOF[ic] + n],
                             wsb[VD][:, kc], start=(kc == 0), stop=False)
        nc.tensor.matmul(pv[:n, :H], onerb[0:1, :n], nbb[0:1, 0],
                         start=False, stop=True)
        nc.vector.tensor_copy(Vd[:n, ic], pv[:n, :H])
        pg = pj()
        for kc in range(2):
            nc.tensor.matmul(pg[:n, :H], DeT[:, kc, DOF[ic]:DOF[ic] + n],
                             wsb[GD][:, kc], start=(kc == 0), stop=False)
        nc.tensor.matmul(pg[:n, :H], onerb[0:1, :n], nbb[0:1, 2],
                         start=False, stop=True)
        nc.scalar.activation(gd[:n, ic], pg[:n, :H], AF.Tanh, scale=0.5)
        nc.vector.tensor_scalar(gd[:n, ic], gd[:n, ic], 0.5, 0.5,
                                op0=ALU.mult, op1=ALU.add)
    Vp = io.tile([128, 8, H], BF16, tag="Vp")
    gp = io.tile([128, 8, H], BF16, tag="gp")
    for jc in range(NJC):
        n = PJN[jc]
        pv = pj()
        for kc in range(2):
            nc.tensor.matmul(pv[:n, :H],
                             PeT[:, kc, 128 * jc:128 * jc + n],
                             wsb[VP][:, kc], start=(kc == 0), stop=False)
        nc.tensor.matmul(pv[:n, :H], onerb[0:1, :n], nbb[0:1, 1],
                         start=False, stop=True)
        nc.vector.tensor_copy(Vp[:n, jc], pv[:n, :H])
        pg = pj()
        for kc in range(2):
            nc.tensor.matmul(pg[:n, :H],
                             PeT[:, kc, 128 * jc:128 * jc + n],
                             wsb[GP][:, kc], start=(kc == 0), stop=False)
        nc.tensor.matmul(pg[:n, :H], onerb[0:1, :n], nbb[0:1, 3],
                         start=False, stop=True)
        nc.scalar.activation(gp[:n, jc], pg[:n, :H], AF.Tanh, scale=0.5)
        nc.vector.tensor_scalar(gp[:n, jc], gp[:n, jc], 0.5, 0.5,
                                op0=ALU.mult, op1=ALU.add)

    # ---- attention per head-group g ----
    ctxdT_sb, ctxpT_sb, colsc = [], [], []
    dmaq = [nc.sync, nc.scalar]
    for g in range(2):
        Sexp = [at.tile([128, 4, PLP], BF16, tag=f"sx{g}{ic}",
                        name=f"sx{g}{ic}") for ic in range(3)]
        SexpT = at.tile([128, 32, NST], BF16, tag=f"sxT{g}", name=f"sxT{g}")
        for ic in range(3):
            n = DIC[ic]
            for hx in range(2):
                for hp in range(2):
                    p = psc()
                    for q in range(2):
                        h4 = 2 * hp + q
                        nc.tensor.matmul(
                            p[:n, q, :],
                            QdT[32 * h4:32 * h4 + 32, g,
                                DOF[ic]:DOF[ic] + n],
                            KpT[32 * h4:32 * h4 + 32, g,
                                512 * hx:512 * hx + 512],
                            start=True, stop=False,
                            tile_position=(32 * h4, 0))
                        nc.tensor.matmul(
                            p[:n, q, :],
                            KdT[32 * h4:32 * h4 + 32, g,
                                DOF[ic]:DOF[ic] + n],
                            QpT[32 * h4:32 * h4 + 32, g,
                                512 * hx:512 * hx + 512],
                            start=False, stop=True,
                            tile_position=(32 * h4, 0))
                    fdj = 512 if hx == 0 else PL - 512
                    nc.scalar.activation(
                        Sexp[ic][:n, 2 * hp:2 * hp + 2,
                                 512 * hx:512 * hx + fdj],
                        p[:n, :, :fdj], AF.Exp)
            # split each transpose across both queues (halve latency)
            nc.sync.dma_start_transpose(
                SexpT[:, 0:16, SOF[ic]:SOF[ic] + SBLK[ic]],
                Sexp[ic][:SBLK[ic], 0:2, :].rearrange("p a b -> p (a b)"))
            nc.scalar.dma_start_transpose(
                SexpT[:, 16:32, SOF[ic]:SOF[ic] + SBLK[ic]],
                Sexp[ic][:SBLK[ic], 2:4, :].rearrange("p a b -> p (a b)"))

        # rowsums (replicated across 32 rows per head) -> reciprocal
        pr = pcx()
        for h4 in range(4):
            for jc in range(NJC):
                nc.tensor.matmul(pr[32 * h4:32 * h4 + 32, :DL],
                                 onesb[:PJN[jc], :32],
                                 SexpT[:PJN[jc], 8 * h4 + jc, :DL],
                                 start=(jc == 0), stop=(jc == NJC - 1),
                                 tile_position=(0, 32 * h4))
        pcd = pj()
        for jc in range(NJC):
            for h4 in range(4):
                h = 4 * g + h4
                nc.tensor.matmul(pcd[32 * h4:32 * h4 + 32, :DL],
                                 Vp[:PJN[jc], jc, 32 * h:32 * h + 32],
                                 SexpT[:PJN[jc], 8 * h4 + jc, :DL],
                                 start=(jc == 0), stop=(jc == NJC - 1),
                                 tile_position=(0, 32 * h4))
        rbcinv = at.tile([128, NST], BF16, tag=f"rbc{g}", name=f"rbc{g}")
        with nc.allow_low_precision(reason="f32r recip"):
            nc.vector.reciprocal(rbcinv[:, :DL], pr[:, :DL])

        # natural-layout rinv (bf16, broadcast to 32 cols per head)
        rinvb32 = at.tile([128, 3, 4, 32], BF16, tag=f"rb32{g}",
                          name=f"rb32{g}")
        for ic in range(3):
            n = DIC[ic]
            p = ptp()
            nc.tensor.transpose(p[:n, :128],
                                rbcinv[0:128, SOF[ic]:SOF[ic] + n],
                                idb[:128, :128])
            src = p[:n, :].rearrange("p (a b) -> p a b", b=32)[:, 0:4, 0]
            nc.vector.tensor_copy(
                rinvb32[:n, ic],
                src.unsqueeze(2).broadcast_to([n, 4, 32]))

        # ctx_dT normalization on evacuation
        cdT = at.tile([128, NST], F32R, tag=f"cd{g}", name=f"cd{g}")
        with nc.allow_low_precision(reason="f32r ctx"):
            nc.vector.tensor_mul(cdT[:, :DL], pcd[:, :DL], rbcinv[:, :DL])
        ctxdT_sb.append(cdT)

        # rinv-scaled Vd for ctx_p
        VdS = at.tile([128, 3, 4, 32], BF16, tag=f"vds{g}", name=f"vds{g}")
        for ic in range(3):
            n = DIC[ic]
            nc.vector.tensor_mul(
                VdS[:n, ic],
                Vd[:n, ic].rearrange("p (a b) -> p a b",
                                     b=32)[:, 4 * g:4 * g + 4],
                rinvb32[:n, ic])

        # ctx_pT + rinv-weighted colsums, per j-half
        cpT = at.tile([128, 2, 512], BF16, tag=f"cp{g}", name=f"cp{g}")
        csT = at.tile([128, 2, 512], BF16, tag=f"cs{g}", name=f"cs{g}")
        for hx in range(2):
            pcp = pj()
            for ic in range(3):
                n = DIC[ic]
                for h4 in range(4):
                    nc.tensor.matmul(
                        pcp[32 * h4:32 * h4 + 32, :512],
                        VdS[:n, ic, h4],
                        Sexp[ic][:n, h4, 512 * hx:512 * hx + 512],
                        start=(ic == 0), stop=(ic == 2),
                        tile_position=(0, 32 * h4))
            nc.vector.tensor_copy(cpT[:, hx], pcp[:, :512])
            pco = pj()
            for ic in range(3):
                n = DIC[ic]
                for h4 in range(4):
                    nc.tensor.matmul(
                        pco[32 * h4:32 * h4 + 32, :512],
                        rinvb32[:n, ic, h4],
                        Sexp[ic][:n, h4, 512 * hx:512 * hx + 512],
                        start=(ic == 0), stop=(ic == 2),
                        tile_position=(0, 32 * h4))
            nc.vector.tensor_copy(csT[:, hx], pco[:, :512])
        ctxpT_sb.append(cpT)
        colsc.append(csT)

    # ---- output: drug side (out = De + pod * gd_sig) ----
    for ic in range(3):
        n = DIC[ic]
        pod = pj()
        for g in range(2):
            nc.tensor.matmul(pod[:n, :H],
                             ctxdT_sb[g][:, DOF[ic]:DOF[ic] + n],
                             wsb[OD][:, g], start=(g == 0), stop=False)
        nc.tensor.matmul(pod[:n, :H], onerr[0:1, :n], nbrod[0:1, 0],
                         start=False, stop=True)
        tmp = io.tile([128, H], F32, tag="ftmp", name="ftmp")
        nc.vector.tensor_mul(tmp[:n], pod[:n, :H], gd[:n, ic])
        ob = io.tile([128, H], F32, tag="ost", name="ost")
        nc.gpsimd.tensor_add(ob[:n], tmp[:n], De[:n, ic])
        nc.gpsimd.dma_start(out[b, DOF[ic]:DOF[ic] + n], ob[:n])

    # ---- output: protein side (out = Pe + pop * gp_sig * sigp) ----
    for jc in range(NJC):
        n = PJN[jc]
        hx, off = jc // 4, (jc % 4) * 128
        pfc = pj()
        for g in range(2):
            nc.tensor.matmul(pfc[:n, :H], colsc[g][:, hx, off:off + n],
                             wfcb[g], start=(g == 0), stop=False)
        nc.tensor.matmul(pfc[:n, :H], onerb[0:1, :n], nbb[0:1, 6],
                         start=False, stop=True)
        sigp = io.tile([128, H], BF16, tag="sigp", name="sigp")
        nc.scalar.activation(sigp[:n], pfc[:n, :H], AF.Tanh, scale=0.5)
        pop = pj()
        for g in range(2):
            nc.tensor.matmul(pop[:n, :H], ctxpT_sb[g][:, hx, off:off + n],
                             wsb[OP][:, g], start=(g == 0), stop=False)
        nc.tensor.matmul(pop[:n, :H], onerb[0:1, :n], nbb[0:1, 5],
                         start=False, stop=True)
        gg = io.tile([128, H], BF16, tag="gg", name="gg")
        nc.vector.scalar_tensor_tensor(gg[:n], sigp[:n], 0.5, gp[:n, jc],
                                       op0=ALU.mult, op1=ALU.mult)
        gg2 = io.tile([128, H], BF16, tag="gg2", name="gg2")
        nc.vector.scalar_tensor_tensor(gg2[:n], gp[:n, jc], 0.5, gg[:n],
                                       op0=ALU.mult, op1=ALU.add)
        tmp = io.tile([128, H], F32, tag="ftmp", name="ftmp")
        nc.vector.tensor_mul(tmp[:n], pop[:n, :H], gg2[:n])
        ob = io.tile([128, H], F32, tag="ost", name="ost")
        nc.gpsimd.tensor_add(ob[:n], tmp[:n], Pe[:n, jc])
        nc.gpsimd.dma_start(out[b, DL + 128 * jc:DL + 128 * jc + n], ob[:n])


def _in_maps(inputs, host):
    in_maps = []
    for c in range(NCORES):
        m = dict(drug=np.ascontiguousarray(inputs["drug"][c * BPC:(c + 1) * BPC]),
                 prot=np.ascontiguousarray(inputs["protein"][c * BPC:(c + 1) * BPC]),
                 wt=host["wt"], tb=host["tb"], nb=host["nb"],
                 wfcg=host["wfcg"], ped=host["ped"], pep=host["pep"],
                 ident=host["ident"], onesrow=host["onesrow"])
        in_maps.append(m)
    return in_maps


def kernel(**inputs):
    inputs = {k: np.asarray(v) for k, v in inputs.items()}
    host = _host_prep(inputs)
    if "nc" not in _CACHE:
        _CACHE["nc"] = _build()
    nc = _CACHE["nc"]
    res = run_bass_kernel_spmd(nc, _in_maps(inputs, host), list(range(NCORES)))
    return np.concatenate([res.results[c]["out"] for c in range(NCORES)],
                          axis=0)



# revision 5
# speedup vs baseline: 1.0117x; 1.0117x over previous
"""Trainium2 Bass kernel for nn_BAGCA, v2.

Data-parallel over batch (32 -> 8 cores x 4). Natural-layout scores +
exp once; transposed attention matrix via DMA xbar transposes; softmax
normalization folded into replicated-rowsum reciprocal (ctx_d),
rinv-scaled Vd (ctx_p) and rinv-weighted colsums (fc_pd). Sigmoids
computed as 0.5*(1+tanh(x/2)) so ACT stays on the exp table set.
bf16 projection path throughout.
"""
import math
import numpy as np

import concourse.bacc as bacc
import concourse.mybir as mybir
import concourse.tile as tile
from concourse.bass_utils import run_bass_kernel_spmd

F32 = mybir.dt.float32
F32R = mybir.dt.float32r
BF16 = mybir.dt.bfloat16
AF = mybir.ActivationFunctionType
ALU = mybir.AluOpType

H = 256
B = 32
DL = 290
PL = 1000
PLP = 1024
NCORES = 8
BPC = B // NCORES
DIC = [128, 128, 34]
DOF = [0, 128, 256]
SOF = [0, 128, 256]
SBLK = [128, 128, 48]
NST = 304
NJC = 8
PJN = [128] * 7 + [104]

_CACHE = {}


def _pe_table(length, d):
    pos = np.arange(length, dtype=np.float32)[:, None]
    div = np.exp(np.arange(0, d, 2, dtype=np.float32)
                 * np.float32(-math.log(10000.0) / d))
    ang = pos * div
    pe = np.zeros((length, d), np.float32)
    pe[:, 0::2] = np.sin(ang)
    pe[:, 1::2] = np.cos(ang)
    return pe


def _sigmoid(x):
    return 1.0 / (1.0 + np.exp(-x))


def _host_prep(inp):
    a = float(_sigmoid(inp["alpha"]))
    isq = 1.0 / math.sqrt(32)
    wqd = inp["w_q_d"] * np.float32(a * isq)
    bqd = inp["b_q_d"] * np.float32(a * isq)
    wqp = inp["w_q_p"] * np.float32((1.0 - a) * isq)
    bqp = inp["b_q_p"] * np.float32((1.0 - a) * isq)

    drug_attn = (1.0 / PL) * inp["w_fc_dp"].sum(0) + inp["b_fc_dp"]
    sig_dp = _sigmoid(drug_attn).astype(np.float32)
    woutd = inp["w_out_d"] * sig_dp[None, :]
    boutd = inp["b_out_d"] * sig_dp

    ws = [wqd, inp["w_k_d"], inp["w_k_p"], wqp, inp["w_v_d"], inp["w_v_p"],
          inp["w_gd"], inp["w_gp"], woutd, inp["w_out_p"]]
    wt = np.stack([w.reshape(2, 128, H) for w in ws]).astype(np.float32)

    tbs = [bqd, inp["b_k_d"], inp["b_k_p"], bqp]
    tb = np.stack([b.reshape(2, 128, 1) for b in tbs]).astype(np.float32)

    nb = np.stack([inp["b_v_d"], inp["b_v_p"], inp["b_gd"], inp["b_gp"],
                   boutd, inp["b_out_p"], inp["b_fc_pd"]])[None].astype(
                       np.float32)

    wfcg = np.zeros((2, 128, H), np.float32)
    for g in range(2):
        for h4 in range(4):
            wfcg[g, 32 * h4, :] = inp["w_fc_pd"][4 * g + h4] / DL

    ped = (inp["scale_d"].reshape(-1)[0] * _pe_table(DL, H)).astype(np.float32)
    pep = (inp["scale_p"].reshape(-1)[0] * _pe_table(PL, H)).astype(np.float32)

    ident = np.eye(128, dtype=np.float32)
    onesrow = np.ones((1, 512), np.float32)
    return dict(wt=wt, tb=tb, nb=nb, wfcg=wfcg, ped=ped, pep=pep,
                ident=ident, onesrow=onesrow)


def _build():
    nc = bacc.Bacc("TRN2", target_bir_lowering=False, debug=False,
                   num_devices=NCORES)
    drug = nc.dram_tensor("drug", [BPC, DL, H], F32, kind="ExternalInput")
    prot = nc.dram_tensor("prot", [BPC, PL, H], F32, kind="ExternalInput")
    wt_d = nc.dram_tensor("wt", [10, 2, 128, H], F32, kind="ExternalInput")
    tb_d = nc.dram_tensor("tb", [4, 2, 128, 1], F32, kind="ExternalInput")
    nb_d = nc.dram_tensor("nb", [1, 7, H], F32, kind="ExternalInput")
    wfcg_d = nc.dram_tensor("wfcg", [2, 128, H], F32, kind="ExternalInput")
    ped_d = nc.dram_tensor("ped", [DL, H], F32, kind="ExternalInput")
    pep_d = nc.dram_tensor("pep", [PL, H], F32, kind="ExternalInput")
    id_d = nc.dram_tensor("ident", [128, 128], F32, kind="ExternalInput")
    or_d = nc.dram_tensor("onesrow", [1, 512], F32, kind="ExternalInput")
    out = nc.dram_tensor("out", [BPC, DL + PL, H], F32, kind="ExternalOutput")

    from contextlib import ExitStack
    with tile.TileContext(nc) as tc, ExitStack() as stack:
        cp = stack.enter_context(tc.tile_pool(name="const", bufs=1))
        io = stack.enter_context(tc.tile_pool(name="io", bufs=2))
        at = stack.enter_context(tc.tile_pool(name="attn", bufs=2))
        a1 = stack.enter_context(tc.tile_pool(name="attn1", bufs=1))
        ps = stack.enter_context(tc.tile_pool(name="ps", bufs=1, space="PSUM"))

        # ---- constants ----
        stg = cp.tile([128, 2, H], F32, tag="stg")
        wsb = []          # QK/V/G weights in bf16; OD f32r; OP bf16
        for w in range(8):
            t = cp.tile([128, 2, H], BF16, tag=f"w{w}", name=f"w{w}")
            nc.sync.dma_start(stg[:], wt_d[w])
            nc.vector.tensor_copy(t[:], stg[:])
            wsb.append(t)
        wod = cp.tile([128, 2, H], F32R, tag="wod")
        nc.sync.dma_start(stg[:], wt_d[8])
        nc.vector.tensor_copy(wod[:], stg[:])
        wsb.append(wod)
        wopb = cp.tile([128, 2, H], BF16, tag="wopb")
        nc.sync.dma_start(stg[:], wt_d[9])
        nc.vector.tensor_copy(wopb[:], stg[:])
        wsb.append(wopb)
        wfcb = []
        for g in range(2):
            t = cp.tile([128, H], BF16, tag=f"wfcb{g}", name=f"wfcb{g}")
            nc.sync.dma_start(stg[:, 0], wfcg_d[g])
            nc.vector.tensor_copy(t[:], stg[:, 0])
            wfcb.append(t)
        tbs = []
        for p in range(4):
            t = cp.tile([128, 2, 1], F32, tag=f"tb{p}", name=f"tb{p}")
            nc.sync.dma_start(t[:], tb_d[p])
            tbs.append(t)
        nbf = cp.tile([1, 7, H], F32, tag="nbf")
        nc.sync.dma_start(nbf[:], nb_d[:])
        nbb = cp.tile([1, 7, H], BF16, tag="nbb")
        nc.vector.tensor_copy(nbb[:], nbf[:])
        nbrod = cp.tile([1, 1, H], F32R, tag="nbrod")
        nc.vector.tensor_copy(nbrod[:], nbf[:, 4:5])
        idf = cp.tile([128, 128], F32, tag="idf")
        nc.sync.dma_start(idf[:], id_d[:])
        idr = cp.tile([128, 128], F32R, tag="idr")
        nc.vector.tensor_copy(idr[:], idf[:])
        idb = cp.tile([128, 128], BF16, tag="idb")
        nc.vector.tensor_copy(idb[:], idf[:])
        onef = cp.tile([1, 512], F32, tag="onef")
        nc.sync.dma_start(onef[:], or_d[:])
        onerr = cp.tile([1, 512], F32R, tag="onerr")
        nc.vector.tensor_copy(onerr[:], onef[:])
        onesb = cp.tile([128, 32], BF16, tag="onesb")
        nc.vector.memset(onesb[:], 1.0)
        onerb = cp.tile([1, 512], BF16, tag="onerb")
        nc.vector.memset(onerb[:], 1.0)
        ped_t = cp.tile([128, 3, H], BF16, tag="ped_t")
        nc.sync.dma_start(stg[:, 0:2, :],
                          ped_d[0:256].rearrange("(c p) h -> p c h", p=128))
        nc.vector.tensor_copy(ped_t[:, 0:2], stg[:])
        nc.sync.dma_start(stg[:34, 0, :], ped_d[256:290])
        nc.vector.tensor_copy(ped_t[:34, 2], stg[:34, 0])
        pep_t = cp.tile([128, 8, H], BF16, tag="pep_t")
        for k in range(3):
            nc.sync.dma_start(
                stg[:, 0:2, :],
                pep_d[256 * k:256 * (k + 1)].rearrange("(c p) h -> p c h",
                                                       p=128))
            nc.vector.tensor_copy(pep_t[:, 2 * k:2 * k + 2], stg[:])
        nc.sync.dma_start(stg[:, 0, :], pep_d[768:896])
        nc.vector.tensor_copy(pep_t[:, 6], stg[:, 0])
        nc.sync.dma_start(stg[:104, 0, :], pep_d[896:1000])
        nc.vector.tensor_copy(pep_t[:104, 7], stg[:104, 0])

        for b in range(BPC):
            _bel(nc, io, at, a1, ps, b, drug, prot, out, wsb, wfcb, tbs, nbb,
                 nbrod, idr, idb, onerr, onesb, onerb, ped_t, pep_t)

    nc.compile()
    return nc


def _bel(nc, io, at, a1, ps, b, drug, prot, out, wsb, wfcb, tbs, nbb,
         nbrod, idr, idb, onerr, onesb, onerb, ped_t, pep_t):
    QD, KD, KP, QP, VD, VP, GD, GP, OD, OP = range(10)

    def pj():
        return ps.tile([128, 512], F32, tag="pj", name="pj", bufs=2)

    def ptp():
        return ps.tile([128, 512], BF16, tag="ptb", name="ptb", bufs=1)

    def psc():
        return ps.tile([128, 2, 512], F32, tag="psc", name="psc", bufs=2)

    def pcx():
        return ps.tile([128, 512], F32, tag="pcx", name="pcx", bufs=1)

    # ---- load inputs (staged) + positional encoding -> bf16 Xe ----
    De = io.tile([128, 3, H], BF16, tag="De")
    Pe = io.tile([128, 8, H], BF16, tag="Pe")
    ldst = io.tile([128, 2, H], F32, tag="ldst")
    nc.gpsimd.dma_start(ldst[:],
                      drug[b, 0:256].rearrange("(c p) h -> p c h", p=128))
    for ic in range(2):
        nc.vector.tensor_add(De[:, ic], ldst[:, ic], ped_t[:, ic])
    ldst2 = io.tile([128, 2, H], F32, tag="ldst2")
    nc.gpsimd.dma_start(ldst2[:34, 0, :], drug[b, 256:290])
    nc.vector.tensor_add(De[:34, 2], ldst2[:34, 0], ped_t[:34, 2])
    for k in range(3):
        st = io.tile([128, 2, H], F32, tag="ldst" if k % 2 == 0 else "ldst2",
                     name="pst")
        nc.gpsimd.dma_start(
            st[:], prot[b, 256 * k:256 * (k + 1)].rearrange(
                "(c p) h -> p c h", p=128))
        for q in range(2):
            nc.vector.tensor_add(Pe[:, 2 * k + q], st[:, q],
                                 pep_t[:, 2 * k + q])
    st = io.tile([128, 2, H], F32, tag="ldst2", name="pst2")
    nc.gpsimd.dma_start(st[:, 0, :], prot[b, 768:896])
    nc.gpsimd.dma_start(st[:104, 1, :], prot[b, 896:1000])
    nc.vector.tensor_add(Pe[:, 6], st[:, 0], pep_t[:, 6])
    nc.vector.tensor_add(Pe[:104, 7], st[:104, 1], pep_t[:104, 7])

    # ---- transpose inputs (bf16) ----
    DeT = a1.tile([128, 2, DL], BF16, tag="DeT")
    for chb in range(2):
        p = ptp()
        for ic in range(3):
            nc.tensor.transpose(p[:, DOF[ic]:DOF[ic] + DIC[ic]],
                                De[:DIC[ic], ic, 128 * chb:128 * chb + 128],
                                idb[:DIC[ic], :DIC[ic]])
        nc.vector.tensor_copy(DeT[:, chb, :], p[:, :DL])
    PeT = a1.tile([128, 2, PLP], BF16, tag="PeT")
    for chb in range(2):
        for hf in range(2):
            p = ptp()
            for k in range(4):
                jc = 4 * hf + k
                nc.tensor.transpose(
                    p[:, 128 * k:128 * k + PJN[jc]],
                    Pe[:PJN[jc], jc, 128 * chb:128 * chb + 128],
                    idb[:PJN[jc], :PJN[jc]])
            nc.vector.tensor_copy(PeT[:, chb, hf * 512:hf * 512 + 512],
                                  p[:, :512])

    # ---- transposed Q/K projections -> bf16 ----
    QdT = a1.tile([128, 2, DL], BF16, tag="QdT")
    KdT = a1.tile([128, 2, DL], BF16, tag="KdT")
    KpT = a1.tile([128, 2, PLP], BF16, tag="KpT")
    QpT = a1.tile([128, 2, PLP], BF16, tag="QpT")
    specs = [(QD, DeT, [(0, DL)], QdT, 0), (KD, DeT, [(0, DL)], KdT, 1),
             (KP, PeT, [(0, 512), (512, 512)], KpT, 2),
             (QP, PeT, [(0, 512), (512, 512)], QpT, 3)]
    for w, xT, halves, outT, tbi in specs:
        for mc in range(2):
            for off, ln in halves:
                p = pj()
                for kc in range(2):
                    nc.tensor.matmul(
                        p[:, :ln], wsb[w][:, kc, 128 * mc:128 * mc + 128],
                        xT[:, kc, off:off + ln],
                        start=(kc == 0), stop=(kc == 1))
                nc.vector.tensor_scalar_add(outT[:, mc, off:off + ln],
                                            p[:, :ln], tbs[tbi][:, mc])
    nc.vector.memset(KpT[:, :, PL:PLP], 0.0)
    nc.vector.memset(QpT[:, :, PL:PLP], 0.0)

    # ---- V / gate projections (natural, bf16); gates as sigmoids ----
    Vd = io.tile([128, 3, H], BF16, tag="Vd")
    gd = io.tile([128, 3, H], BF16, tag="gd")
    for ic in range(3):
        n = DIC[ic]
        pv = pj()
        for kc in range(2):
            nc.tensor.matmul(pv[:n, :H], DeT[:, kc, DOF[ic]:DOF[ic] + n],
                             wsb[VD][:, kc], start=(kc == 0), stop=False)
        nc.tensor.matmul(pv[:n, :H], onerb[0:1, :n], nbb[0:1, 0],
                         start=False, stop=True)
        nc.vector.tensor_copy(Vd[:n, ic], pv[:n, :H])
        pg = pj()
        for kc in range(2):
            nc.tensor.matmul(pg[:n, :H], DeT[:, kc, DOF[ic]:DOF[ic] + n],
                             wsb[GD][:, kc], start=(kc == 0), stop=False)
        nc.tensor.matmul(pg[:n, :H], onerb[0:1, :n], nbb[0:1, 2],
                         start=False, stop=True)
        nc.scalar.activation(gd[:n, ic], pg[:n, :H], AF.Tanh, scale=0.5)
        nc.vector.tensor_scalar(gd[:n, ic], gd[:n, ic], 0.5, 0.5,
                                op0=ALU.mult, op1=ALU.add)
    Vp = io.tile([128, 8, H], BF16, tag="Vp")
    gp = io.tile([128, 8, H], BF16, tag="gp")
    for jc in range(NJC):
        n = PJN[jc]
        pv = pj()
        for kc in range(2):
            nc.tensor.matmul(pv[:n, :H],
                             PeT[:, kc, 128 * jc:128 * jc + n],
                             wsb[VP][:, kc], start=(kc == 0), stop=False)
        nc.tensor.matmul(pv[:n, :H], onerb[0:1, :n], nbb[0:1, 1],
                         start=False, stop=True)
        nc.vector.tensor_copy(Vp[:n, jc], pv[:n, :H])
        pg = pj()
        for kc in range(2):
            nc.tensor.matmul(pg[:n, :H],
                             PeT[:, kc, 128 * jc:128 * jc + n],
                             wsb[GP][:, kc], start=(kc == 0), stop=False)
        nc.tensor.matmul(pg[:n, :H], onerb[0:1, :n], nbb[0:1, 3],
                         start=False, stop=True)
        nc.scalar.activation(gp[:n, jc], pg[:n, :H], AF.Tanh, scale=0.5)
        nc.vector.tensor_scalar(gp[:n, jc], gp[:n, jc], 0.5, 0.5,
                                op0=ALU.mult, op1=ALU.add)

    # ---- attention per head-group g ----
    ctxdT_sb, ctxpT_sb, colsc = [], [], []
    dmaq = [nc.sync, nc.scalar]
    for g in range(2):
        Sexp = [at.tile([128, 4, PLP], BF16, tag=f"sx{ic}",
                        name=f"sx{g}{ic}") for ic in range(3)]
        SexpT = a1.tile([128, 32, NST], BF16, tag="sxT", name=f"sxT{g}")
        for ic in range(3):
            n = DIC[ic]
            for hx in range(2):
                for hp in range(2):
                    p = psc()
                    for q in range(2):
                        h4 = 2 * hp + q
                        nc.tensor.matmul(
                            p[:n, q, :],
                            QdT[32 * h4:32 * h4 + 32, g,
                                DOF[ic]:DOF[ic] + n],
                            KpT[32 * h4:32 * h4 + 32, g,
                                512 * hx:512 * hx + 512],
                            start=True, stop=False,
                            tile_position=(32 * h4, 0))
                        nc.tensor.matmul(
                            p[:n, q, :],
                            KdT[32 * h4:32 * h4 + 32, g,
                                DOF[ic]:DOF[ic] + n],
                            QpT[32 * h4:32 * h4 + 32, g,
                                512 * hx:512 * hx + 512],
                            start=False, stop=True,
                            tile_position=(32 * h4, 0))
                    fdj = 512 if hx == 0 else PL - 512
                    nc.scalar.activation(
                        Sexp[ic][:n, 2 * hp:2 * hp + 2,
                                 512 * hx:512 * hx + fdj],
                        p[:n, :, :fdj], AF.Exp)
            # split each transpose across both queues (halve latency)
            nc.sync.dma_start_transpose(
                SexpT[:, 0:16, SOF[ic]:SOF[ic] + SBLK[ic]],
                Sexp[ic][:SBLK[ic], 0:2, :].rearrange("p a b -> p (a b)"))
            nc.scalar.dma_start_transpose(
                SexpT[:, 16:32, SOF[ic]:SOF[ic] + SBLK[ic]],
                Sexp[ic][:SBLK[ic], 2:4, :].rearrange("p a b -> p (a b)"))

        # rowsums (replicated across 32 rows per head) -> reciprocal
        pr = pcx()
        for h4 in range(4):
            for jc in range(NJC):
                nc.tensor.matmul(pr[32 * h4:32 * h4 + 32, :DL],
                                 onesb[:PJN[jc], :32],
                                 SexpT[:PJN[jc], 8 * h4 + jc, :DL],
                                 start=(jc == 0), stop=(jc == NJC - 1),
                                 tile_position=(0, 32 * h4))
        pcd = pj()
        for jc in range(NJC):
            for h4 in range(4):
                h = 4 * g + h4
                nc.tensor.matmul(pcd[32 * h4:32 * h4 + 32, :DL],
                                 Vp[:PJN[jc], jc, 32 * h:32 * h + 32],
                                 SexpT[:PJN[jc], 8 * h4 + jc, :DL],
                                 start=(jc == 0), stop=(jc == NJC - 1),
                                 tile_position=(0, 32 * h4))
        rbcinv = at.tile([128, NST], BF16, tag=f"rbc{g}", name=f"rbc{g}")
        with nc.allow_low_precision(reason="f32r recip"):
            nc.vector.reciprocal(rbcinv[:, :DL], pr[:, :DL])

        # natural-layout rinv (bf16, broadcast to 32 cols per head)
        rinvb32 = at.tile([128, 3, 4, 32], BF16, tag=f"rb32{g}",
                          name=f"rb32{g}")
        for ic in range(3):
            n = DIC[ic]
            p = ptp()
            nc.tensor.transpose(p[:n, :128],
                                rbcinv[0:128, SOF[ic]:SOF[ic] + n],
                                idb[:128, :128])
            src = p[:n, :].rearrange("p (a b) -> p a b", b=32)[:, 0:4, 0]
            nc.vector.tensor_copy(
                rinvb32[:n, ic],
                src.unsqueeze(2).broadcast_to([n, 4, 32]))

        # ctx_dT normalization on evacuation
        cdT = at.tile([128, NST], F32R, tag=f"cd{g}", name=f"cd{g}")
        with nc.allow_low_precision(reason="f32r ctx"):
            nc.vector.tensor_mul(cdT[:, :DL], pcd[:, :DL], rbcinv[:, :DL])
        ctxdT_sb.append(cdT)

        # rinv-scaled Vd for ctx_p
        VdS = at.tile([128, 3, 4, 32], BF16, tag=f"vds{g}", name=f"vds{g}")
        for ic in range(3):
            n = DIC[ic]
            nc.vector.tensor_mul(
                VdS[:n, ic],
                Vd[:n, ic].rearrange("p (a b) -> p a b",
                                     b=32)[:, 4 * g:4 * g + 4],
                rinvb32[:n, ic])

        # ctx_pT + rinv-weighted colsums, per j-half
        cpT = at.tile([128, 2, 512], BF16, tag=f"cp{g}", name=f"cp{g}")
        csT = at.tile([128, 2, 512], BF16, tag=f"cs{g}", name=f"cs{g}")
        for hx in range(2):
            pcp = pj()
            for ic in range(3):
                n = DIC[ic]
                for h4 in range(4):
                    nc.tensor.matmul(
                        pcp[32 * h4:32 * h4 + 32, :512],
                        VdS[:n, ic, h4],
                        Sexp[ic][:n, h4, 512 * hx:512 * hx + 512],
                        start=(ic == 0), stop=(ic == 2),
                        tile_position=(0, 32 * h4))
            nc.vector.tensor_copy(cpT[:, hx], pcp[:, :512])
            pco = pj()
            for ic in range(3):
                n = DIC[ic]
                for h4 in range(4):
                    nc.tensor.matmul(
                        pco[32 * h4:32 * h4 + 32, :512],
                        rinvb32[:n, ic, h4],
                        Sexp[ic][:n, h4, 512 * hx:512 * hx + 512],
                        start=(ic == 0), stop=(ic == 2),
                        tile_position=(0, 32 * h4))
            nc.vector.tensor_copy(csT[:, hx], pco[:, :512])
        ctxpT_sb.append(cpT)
        colsc.append(csT)

    # ---- output: drug side (out = De + pod * gd_sig) ----
    for ic in range(3):
        n = DIC[ic]
        pod = pj()
        for g in range(2):
            nc.tensor.matmul(pod[:n, :H],
                             ctxdT_sb[g][:, DOF[ic]:DOF[ic] + n],
                             wsb[OD][:, g], start=(g == 0), stop=False)
        nc.tensor.matmul(pod[:n, :H], onerr[0:1, :n], nbrod[0:1, 0],
                         start=False, stop=True)
        tmp = io.tile([128, H], F32, tag="ftmp", name="ftmp")
        nc.vector.tensor_mul(tmp[:n], pod[:n, :H], gd[:n, ic])
        ob = io.tile([128, H], F32, tag="ost", name="ost")
        nc.gpsimd.tensor_add(ob[:n], tmp[:n], De[:n, ic])
        nc.gpsimd.dma_start(out[b, DOF[ic]:DOF[ic] + n], ob[:n])

    # ---- output: protein side (out = Pe + pop * gp_sig * sigp) ----
    for jc in range(NJC):
        n = PJN[jc]
        hx, off = jc // 4, (jc % 4) * 128
        pfc = pj()
        for g in range(2):
            nc.tensor.matmul(pfc[:n, :H], colsc[g][:, hx, off:off + n],
                             wfcb[g], start=(g == 0), stop=False)
        nc.tensor.matmul(pfc[:n, :H], onerb[0:1, :n], nbb[0:1, 6],
                         start=False, stop=True)
        sigp = io.tile([128, H], BF16, tag="sigp", name="sigp")
        nc.scalar.activation(sigp[:n], pfc[:n, :H], AF.Tanh, scale=0.5)
        pop = pj()
        for g in range(2):
            nc.tensor.matmul(pop[:n, :H], ctxpT_sb[g][:, hx, off:off + n],
                             wsb[OP][:, g], start=(g == 0), stop=False)
        nc.tensor.matmul(pop[:n, :H], onerb[0:1, :n], nbb[0:1, 5],
                         start=False, stop=True)
        gg = io.tile([128, H], BF16, tag="gg", name="gg")
        nc.vector.scalar_tensor_tensor(gg[:n], sigp[:n], 0.5, gp[:n, jc],
                                       op0=ALU.mult, op1=ALU.mult)
        gg2 = io.tile([128, H], BF16, tag="gg2", name="gg2")
        nc.vector.scalar_tensor_tensor(gg2[:n], gp[:n, jc], 0.5, gg[:n],
                                       op0=ALU.mult, op1=ALU.add)
        tmp = io.tile([128, H], F32, tag="ftmp", name="ftmp")
        nc.vector.tensor_mul(tmp[:n], pop[:n, :H], gg2[:n])
        ob = io.tile([128, H], F32, tag="ost", name="ost")
        nc.gpsimd.tensor_add(ob[:n], tmp[:n], Pe[:n, jc])
        nc.gpsimd.dma_start(out[b, DL + 128 * jc:DL + 128 * jc + n], ob[:n])


def _in_maps(inputs, host):
    in_maps = []
    for c in range(NCORES):
        m = dict(drug=np.ascontiguousarray(inputs["drug"][c * BPC:(c + 1) * BPC]),
                 prot=np.ascontiguousarray(inputs["protein"][c * BPC:(c + 1) * BPC]),
                 wt=host["wt"], tb=host["tb"], nb=host["nb"],
                 wfcg=host["wfcg"], ped=host["ped"], pep=host["pep"],
                 ident=host["ident"], onesrow=host["onesrow"])
        in_maps.append(m)
    return in_maps


def kernel(**inputs):
    inputs = {k: np.asarray(v) for k, v in inputs.items()}
    host = _host_prep(inputs)
    if "nc" not in _CACHE:
        _CACHE["nc"] = _build()
    nc = _CACHE["nc"]
    res = run_bass_kernel_spmd(nc, _in_maps(inputs, host), list(range(NCORES)))
    return np.concatenate([res.results[c]["out"] for c in range(NCORES)],
                          axis=0)



# revision 6
# speedup vs baseline: 1.1119x; 1.0991x over previous
"""Trainium2 Bass kernel for nn_BAGCA, v2.

Data-parallel over batch (32 -> 8 cores x 4). Natural-layout scores +
exp once; transposed attention matrix via DMA xbar transposes; softmax
normalization folded into replicated-rowsum reciprocal (ctx_d),
rinv-scaled Vd (ctx_p) and rinv-weighted colsums (fc_pd). Sigmoids
computed as 0.5*(1+tanh(x/2)) so ACT stays on the exp table set.
bf16 projection path throughout.
"""
import math
import numpy as np

import concourse.bacc as bacc
import concourse.mybir as mybir
import concourse.tile as tile
from concourse.bass_utils import run_bass_kernel_spmd

F32 = mybir.dt.float32
F32R = mybir.dt.float32r
BF16 = mybir.dt.bfloat16
AF = mybir.ActivationFunctionType
ALU = mybir.AluOpType

H = 256
B = 32
DL = 290
PL = 1000
PLP = 1024
NCORES = 8
BPC = B // NCORES
DIC = [128, 128, 34]
DOF = [0, 128, 256]
SOF = [0, 128, 256]
SBLK = [128, 128, 48]
NST = 304
NJC = 8
PJN = [128] * 7 + [104]

_CACHE = {}


def _pe_table(length, d):
    pos = np.arange(length, dtype=np.float32)[:, None]
    div = np.exp(np.arange(0, d, 2, dtype=np.float32)
                 * np.float32(-math.log(10000.0) / d))
    ang = pos * div
    pe = np.zeros((length, d), np.float32)
    pe[:, 0::2] = np.sin(ang)
    pe[:, 1::2] = np.cos(ang)
    return pe


def _sigmoid(x):
    return 1.0 / (1.0 + np.exp(-x))


def _host_prep(inp):
    a = float(_sigmoid(inp["alpha"]))
    isq = 1.0 / math.sqrt(32)
    wqd = inp["w_q_d"] * np.float32(a * isq)
    bqd = inp["b_q_d"] * np.float32(a * isq)
    wqp = inp["w_q_p"] * np.float32((1.0 - a) * isq)
    bqp = inp["b_q_p"] * np.float32((1.0 - a) * isq)

    drug_attn = (1.0 / PL) * inp["w_fc_dp"].sum(0) + inp["b_fc_dp"]
    sig_dp = _sigmoid(drug_attn).astype(np.float32)
    woutd = inp["w_out_d"] * sig_dp[None, :]
    boutd = inp["b_out_d"] * sig_dp

    ws = [wqd, inp["w_k_d"], inp["w_k_p"], wqp, inp["w_v_d"], inp["w_v_p"],
          inp["w_gd"], inp["w_gp"], woutd, inp["w_out_p"]]
    wt = np.stack([w.reshape(2, 128, H) for w in ws]).astype(np.float32)

    tbs = [bqd, inp["b_k_d"], inp["b_k_p"], bqp]
    tb = np.stack([b.reshape(2, 128, 1) for b in tbs]).astype(np.float32)

    nb = np.stack([inp["b_v_d"], inp["b_v_p"], inp["b_gd"], inp["b_gp"],
                   boutd, inp["b_out_p"], inp["b_fc_pd"]])[None].astype(
                       np.float32)

    wfcg = np.zeros((2, 128, H), np.float32)
    for g in range(2):
        for h4 in range(4):
            wfcg[g, 32 * h4, :] = inp["w_fc_pd"][4 * g + h4] / DL

    ped = (inp["scale_d"].reshape(-1)[0] * _pe_table(DL, H)).astype(np.float32)
    pep = (inp["scale_p"].reshape(-1)[0] * _pe_table(PL, H)).astype(np.float32)

    ident = np.eye(128, dtype=np.float32)
    onesrow = np.ones((1, 512), np.float32)
    return dict(wt=wt, tb=tb, nb=nb, wfcg=wfcg, ped=ped, pep=pep,
                ident=ident, onesrow=onesrow)


def _build():
    nc = bacc.Bacc("TRN2", target_bir_lowering=False, debug=False,
                   num_devices=NCORES)
    drug = nc.dram_tensor("drug", [BPC, DL, H], F32, kind="ExternalInput")
    prot = nc.dram_tensor("prot", [BPC, PL, H], F32, kind="ExternalInput")
    wt_d = nc.dram_tensor("wt", [10, 2, 128, H], F32, kind="ExternalInput")
    tb_d = nc.dram_tensor("tb", [4, 2, 128, 1], F32, kind="ExternalInput")
    nb_d = nc.dram_tensor("nb", [1, 7, H], F32, kind="ExternalInput")
    wfcg_d = nc.dram_tensor("wfcg", [2, 128, H], F32, kind="ExternalInput")
    ped_d = nc.dram_tensor("ped", [DL, H], F32, kind="ExternalInput")
    pep_d = nc.dram_tensor("pep", [PL, H], F32, kind="ExternalInput")
    id_d = nc.dram_tensor("ident", [128, 128], F32, kind="ExternalInput")
    or_d = nc.dram_tensor("onesrow", [1, 512], F32, kind="ExternalInput")
    out = nc.dram_tensor("out", [BPC, DL + PL, H], F32, kind="ExternalOutput")

    from contextlib import ExitStack
    with tile.TileContext(nc) as tc, ExitStack() as stack:
        cp = stack.enter_context(tc.tile_pool(name="const", bufs=1))
        io = stack.enter_context(tc.tile_pool(name="io", bufs=2))
        at = stack.enter_context(tc.tile_pool(name="attn", bufs=2))
        a1 = stack.enter_context(tc.tile_pool(name="attn1", bufs=1))
        ps = stack.enter_context(tc.tile_pool(name="ps", bufs=1, space="PSUM"))

        # ---- constants ----
        stgs = [cp.tile([128, 2, H], F32, tag=f"stg{i}", name=f"stg{i}")
                for i in range(4)]
        dmae = [nc.sync, nc.scalar, nc.gpsimd]
        nst = [0]
        def stg_dma(dst_shape_slice, src):
            st = stgs[nst[0] % 4]
            eng = dmae[nst[0] % 3]
            nst[0] += 1
            eng.dma_start(st[dst_shape_slice] if dst_shape_slice else st[:], src)
            return st
        wsb = []          # QK/V/G weights in bf16; OD f32r; OP bf16
        for w in range(8):
            t = cp.tile([128, 2, H], BF16, tag=f"w{w}", name=f"w{w}")
            st = stg_dma(None, wt_d[w])
            nc.vector.tensor_copy(t[:], st[:])
            wsb.append(t)
        wod = cp.tile([128, 2, H], F32R, tag="wod")
        st = stg_dma(None, wt_d[8])
        nc.vector.tensor_copy(wod[:], st[:])
        wsb.append(wod)
        wopb = cp.tile([128, 2, H], BF16, tag="wopb")
        st = stg_dma(None, wt_d[9])
        nc.vector.tensor_copy(wopb[:], st[:])
        wsb.append(wopb)
        wfcb = []
        for g in range(2):
            t = cp.tile([128, H], BF16, tag=f"wfcb{g}", name=f"wfcb{g}")
            st = stg_dma((slice(None), 0), wfcg_d[g])
            nc.vector.tensor_copy(t[:], st[:, 0])
            wfcb.append(t)
        tbs = []
        for p in range(4):
            t = cp.tile([128, 2, 1], F32, tag=f"tb{p}", name=f"tb{p}")
            nc.sync.dma_start(t[:], tb_d[p])
            tbs.append(t)
        nbf = cp.tile([1, 7, H], F32, tag="nbf")
        nc.sync.dma_start(nbf[:], nb_d[:])
        nbb = cp.tile([1, 7, H], BF16, tag="nbb")
        nc.vector.tensor_copy(nbb[:], nbf[:])
        nbrod = cp.tile([1, 1, H], F32R, tag="nbrod")
        nc.vector.tensor_copy(nbrod[:], nbf[:, 4:5])
        idf = cp.tile([128, 128], F32, tag="idf")
        nc.sync.dma_start(idf[:], id_d[:])
        idr = cp.tile([128, 128], F32R, tag="idr")
        nc.vector.tensor_copy(idr[:], idf[:])
        idb = cp.tile([128, 128], BF16, tag="idb")
        nc.vector.tensor_copy(idb[:], idf[:])
        onef = cp.tile([1, 512], F32, tag="onef")
        nc.sync.dma_start(onef[:], or_d[:])
        onerr = cp.tile([1, 512], F32R, tag="onerr")
        nc.vector.tensor_copy(onerr[:], onef[:])
        onesb = cp.tile([128, 32], BF16, tag="onesb")
        nc.vector.memset(onesb[:], 1.0)
        onerb = cp.tile([1, 512], BF16, tag="onerb")
        nc.vector.memset(onerb[:], 1.0)
        ped_t = cp.tile([128, 3, H], BF16, tag="ped_t")
        nc.sync.dma_start(stgs[0][:, 0:2, :],
                          ped_d[0:256].rearrange("(c p) h -> p c h", p=128))
        nc.vector.tensor_copy(ped_t[:, 0:2], stgs[0][:])
        nc.scalar.dma_start(stgs[1][:34, 0, :], ped_d[256:290])
        nc.vector.tensor_copy(ped_t[:34, 2], stgs[1][:34, 0])
        pep_t = cp.tile([128, 8, H], BF16, tag="pep_t")
        for k in range(3):
            st = stgs[2 + (k % 2)]
            [nc.sync, nc.scalar, nc.gpsimd][k].dma_start(
                st[:, 0:2, :],
                pep_d[256 * k:256 * (k + 1)].rearrange("(c p) h -> p c h",
                                                       p=128))
            nc.vector.tensor_copy(pep_t[:, 2 * k:2 * k + 2], st[:])
        nc.sync.dma_start(stgs[0][:, 0, :], pep_d[768:896])
        nc.vector.tensor_copy(pep_t[:, 6], stgs[0][:, 0])
        nc.scalar.dma_start(stgs[1][:104, 0, :], pep_d[896:1000])
        nc.vector.tensor_copy(pep_t[:104, 7], stgs[1][:104, 0])

        for b in range(BPC):
            _bel(nc, io, at, a1, ps, b, drug, prot, out, wsb, wfcb, tbs, nbb,
                 nbrod, idr, idb, onerr, onesb, onerb, ped_t, pep_t)

    nc.compile()
    return nc


def _bel(nc, io, at, a1, ps, b, drug, prot, out, wsb, wfcb, tbs, nbb,
         nbrod, idr, idb, onerr, onesb, onerb, ped_t, pep_t):
    QD, KD, KP, QP, VD, VP, GD, GP, OD, OP = range(10)

    def pj():
        return ps.tile([128, 512], F32, tag="pj", name="pj", bufs=2)

    def ptp():
        return ps.tile([128, 512], BF16, tag="ptb", name="ptb", bufs=1)

    def psc():
        return ps.tile([128, 2, 512], F32, tag="psc", name="psc", bufs=2)

    def pcx():
        return ps.tile([128, 512], F32, tag="pcx", name="pcx", bufs=1)

    # ---- load inputs (staged) + positional encoding -> bf16 Xe ----
    De = io.tile([128, 3, H], BF16, tag="De")
    Pe = io.tile([128, 8, H], BF16, tag="Pe")
    ldst = io.tile([128, 2, H], F32, tag="ldst")
    nc.gpsimd.dma_start(ldst[:],
                      drug[b, 0:256].rearrange("(c p) h -> p c h", p=128))
    for ic in range(2):
        nc.vector.tensor_add(De[:, ic], ldst[:, ic], ped_t[:, ic])
    ldst2 = io.tile([128, 2, H], F32, tag="ldst2")
    nc.gpsimd.dma_start(ldst2[:34, 0, :], drug[b, 256:290])
    nc.vector.tensor_add(De[:34, 2], ldst2[:34, 0], ped_t[:34, 2])
    for k in range(3):
        st = io.tile([128, 2, H], F32, tag="ldst" if k % 2 == 0 else "ldst2",
                     name="pst")
        nc.gpsimd.dma_start(
            st[:], prot[b, 256 * k:256 * (k + 1)].rearrange(
                "(c p) h -> p c h", p=128))
        for q in range(2):
            nc.vector.tensor_add(Pe[:, 2 * k + q], st[:, q],
                                 pep_t[:, 2 * k + q])
    st = io.tile([128, 2, H], F32, tag="ldst2", name="pst2")
    nc.gpsimd.dma_start(st[:, 0, :], prot[b, 768:896])
    nc.gpsimd.dma_start(st[:104, 1, :], prot[b, 896:1000])
    nc.vector.tensor_add(Pe[:, 6], st[:, 0], pep_t[:, 6])
    nc.vector.tensor_add(Pe[:104, 7], st[:104, 1], pep_t[:104, 7])

    # ---- transpose inputs (bf16) ----
    DeT = a1.tile([128, 2, DL], BF16, tag="DeT")
    for chb in range(2):
        p = ptp()
        for ic in range(3):
            nc.tensor.transpose(p[:, DOF[ic]:DOF[ic] + DIC[ic]],
                                De[:DIC[ic], ic, 128 * chb:128 * chb + 128],
                                idb[:DIC[ic], :DIC[ic]])
        nc.vector.tensor_copy(DeT[:, chb, :], p[:, :DL])
    PeT = a1.tile([128, 2, PLP], BF16, tag="PeT")
    for chb in range(2):
        for hf in range(2):
            p = ptp()
            for k in range(4):
                jc = 4 * hf + k
                nc.tensor.transpose(
                    p[:, 128 * k:128 * k + PJN[jc]],
                    Pe[:PJN[jc], jc, 128 * chb:128 * chb + 128],
                    idb[:PJN[jc], :PJN[jc]])
            nc.vector.tensor_copy(PeT[:, chb, hf * 512:hf * 512 + 512],
                                  p[:, :512])

    # ---- transposed Q/K projections -> bf16 ----
    QdT = a1.tile([128, 2, DL], BF16, tag="QdT")
    KdT = a1.tile([128, 2, DL], BF16, tag="KdT")
    KpT = a1.tile([128, 2, PLP], BF16, tag="KpT")
    QpT = a1.tile([128, 2, PLP], BF16, tag="QpT")
    specs = [(QD, DeT, [(0, DL)], QdT, 0), (KD, DeT, [(0, DL)], KdT, 1),
             (KP, PeT, [(0, 512), (512, 512)], KpT, 2),
             (QP, PeT, [(0, 512), (512, 512)], QpT, 3)]
    for w, xT, halves, outT, tbi in specs:
        for mc in range(2):
            for off, ln in halves:
                p = pj()
                for kc in range(2):
                    nc.tensor.matmul(
                        p[:, :ln], wsb[w][:, kc, 128 * mc:128 * mc + 128],
                        xT[:, kc, off:off + ln],
                        start=(kc == 0), stop=(kc == 1))
                nc.vector.tensor_scalar_add(outT[:, mc, off:off + ln],
                                            p[:, :ln], tbs[tbi][:, mc])
    nc.vector.memset(KpT[:, :, PL:PLP], 0.0)
    nc.vector.memset(QpT[:, :, PL:PLP], 0.0)

    # ---- V / gate projections (natural, bf16); gates as sigmoids ----
    Vd = io.tile([128, 3, H], BF16, tag="Vd")
    gd = io.tile([128, 3, H], BF16, tag="gd")
    for ic in range(3):
        n = DIC[ic]
        pv = pj()
        for kc in range(2):
            nc.tensor.matmul(pv[:n, :H], DeT[:, kc, DOF[ic]:DOF[ic] + n],
                             wsb[VD][:, kc], start=(kc == 0), stop=False)
        nc.tensor.matmul(pv[:n, :H], onerb[0:1, :n], nbb[0:1, 0],
                         start=False, stop=True)
        nc.vector.tensor_copy(Vd[:n, ic], pv[:n, :H])
        pg = pj()
        for kc in range(2):
            nc.tensor.matmul(pg[:n, :H], DeT[:, kc, DOF[ic]:DOF[ic] + n],
                             wsb[GD][:, kc], start=(kc == 0), stop=False)
        nc.tensor.matmul(pg[:n, :H], onerb[0:1, :n], nbb[0:1, 2],
                         start=False, stop=True)
        nc.scalar.activation(gd[:n, ic], pg[:n, :H], AF.Tanh, scale=0.5)
        nc.vector.tensor_scalar(gd[:n, ic], gd[:n, ic], 0.5, 0.5,
                                op0=ALU.mult, op1=ALU.add)
    Vp = io.tile([128, 8, H], BF16, tag="Vp")
    gp = io.tile([128, 8, H], BF16, tag="gp")
    for jc in range(NJC):
        n = PJN[jc]
        pv = pj()
        for kc in range(2):
            nc.tensor.matmul(pv[:n, :H],
                             PeT[:, kc, 128 * jc:128 * jc + n],
                             wsb[VP][:, kc], start=(kc == 0), stop=False)
        nc.tensor.matmul(pv[:n, :H], onerb[0:1, :n], nbb[0:1, 1],
                         start=False, stop=True)
        nc.vector.tensor_copy(Vp[:n, jc], pv[:n, :H])
        pg = pj()
        for kc in range(2):
            nc.tensor.matmul(pg[:n, :H],
                             PeT[:, kc, 128 * jc:128 * jc + n],
                             wsb[GP][:, kc], start=(kc == 0), stop=False)
        nc.tensor.matmul(pg[:n, :H], onerb[0:1, :n], nbb[0:1, 3],
                         start=False, stop=True)
        nc.scalar.activation(gp[:n, jc], pg[:n, :H], AF.Tanh, scale=0.5)
        nc.vector.tensor_scalar(gp[:n, jc], gp[:n, jc], 0.5, 0.5,
                                op0=ALU.mult, op1=ALU.add)

    # ---- attention per head-group g ----
    ctxdT_sb, ctxpT_sb, colsc = [], [], []
    dmaq = [nc.sync, nc.scalar]
    for g in range(2):
        Sexp = [at.tile([128, 4, PLP], BF16, tag=f"sx{ic}",
                        name=f"sx{g}{ic}") for ic in range(3)]
        SexpT = a1.tile([128, 32, NST], BF16, tag="sxT", name=f"sxT{g}")
        for ic in range(3):
            n = DIC[ic]
            for hx in range(2):
                for hp in range(2):
                    p = psc()
                    for q in range(2):
                        h4 = 2 * hp + q
                        nc.tensor.matmul(
                            p[:n, q, :],
                            QdT[32 * h4:32 * h4 + 32, g,
                                DOF[ic]:DOF[ic] + n],
                            KpT[32 * h4:32 * h4 + 32, g,
                                512 * hx:512 * hx + 512],
                            start=True, stop=False,
                            tile_position=(32 * h4, 0))
                        nc.tensor.matmul(
                            p[:n, q, :],
                            KdT[32 * h4:32 * h4 + 32, g,
                                DOF[ic]:DOF[ic] + n],
                            QpT[32 * h4:32 * h4 + 32, g,
                                512 * hx:512 * hx + 512],
                            start=False, stop=True,
                            tile_position=(32 * h4, 0))
                    fdj = 512 if hx == 0 else PL - 512
                    nc.scalar.activation(
                        Sexp[ic][:n, 2 * hp:2 * hp + 2,
                                 512 * hx:512 * hx + fdj],
                        p[:n, :, :fdj], AF.Exp)
            # split each transpose across both queues (halve latency)
            nc.sync.dma_start_transpose(
                SexpT[:, 0:16, SOF[ic]:SOF[ic] + SBLK[ic]],
                Sexp[ic][:SBLK[ic], 0:2, :].rearrange("p a b -> p (a b)"))
            nc.scalar.dma_start_transpose(
                SexpT[:, 16:32, SOF[ic]:SOF[ic] + SBLK[ic]],
                Sexp[ic][:SBLK[ic], 2:4, :].rearrange("p a b -> p (a b)"))

        # rowsums (replicated across 32 rows per head) -> reciprocal
        pr = pcx()
        for h4 in range(4):
            for jc in range(NJC):
                nc.tensor.matmul(pr[32 * h4:32 * h4 + 32, :DL],
                                 onesb[:PJN[jc], :32],
                                 SexpT[:PJN[jc], 8 * h4 + jc, :DL],
                                 start=(jc == 0), stop=(jc == NJC - 1),
                                 tile_position=(0, 32 * h4))
        pcd = pj()
        for jc in range(NJC):
            for h4 in range(4):
                h = 4 * g + h4
                nc.tensor.matmul(pcd[32 * h4:32 * h4 + 32, :DL],
                                 Vp[:PJN[jc], jc, 32 * h:32 * h + 32],
                                 SexpT[:PJN[jc], 8 * h4 + jc, :DL],
                                 start=(jc == 0), stop=(jc == NJC - 1),
                                 tile_position=(0, 32 * h4))
        rbcinv = at.tile([128, NST], BF16, tag=f"rbc{g}", name=f"rbc{g}")
        with nc.allow_low_precision(reason="f32r recip"):
            nc.vector.reciprocal(rbcinv[:, :DL], pr[:, :DL])

        # natural-layout rinv (bf16, broadcast to 32 cols per head)
        rinvb32 = at.tile([128, 3, 4, 32], BF16, tag=f"rb32{g}",
                          name=f"rb32{g}")
        for ic in range(3):
            n = DIC[ic]
            p = ptp()
            nc.tensor.transpose(p[:n, :128],
                                rbcinv[0:128, SOF[ic]:SOF[ic] + n],
                                idb[:128, :128])
            src = p[:n, :].rearrange("p (a b) -> p a b", b=32)[:, 0:4, 0]
            nc.vector.tensor_copy(
                rinvb32[:n, ic],
                src.unsqueeze(2).broadcast_to([n, 4, 32]))

        # ctx_dT normalization on evacuation
        cdT = at.tile([128, NST], F32R, tag=f"cd{g}", name=f"cd{g}")
        with nc.allow_low_precision(reason="f32r ctx"):
            nc.vector.tensor_mul(cdT[:, :DL], pcd[:, :DL], rbcinv[:, :DL])
        ctxdT_sb.append(cdT)

        # rinv-scaled Vd for ctx_p
        VdS = at.tile([128, 3, 4, 32], BF16, tag=f"vds{g}", name=f"vds{g}")
        for ic in range(3):
            n = DIC[ic]
            nc.vector.tensor_mul(
                VdS[:n, ic],
                Vd[:n, ic].rearrange("p (a b) -> p a b",
                                     b=32)[:, 4 * g:4 * g + 4],
                rinvb32[:n, ic])

        # ctx_pT + rinv-weighted colsums, per j-half
        cpT = at.tile([128, 2, 512], BF16, tag=f"cp{g}", name=f"cp{g}")
        csT = at.tile([128, 2, 512], BF16, tag=f"cs{g}", name=f"cs{g}")
        for hx in range(2):
            pcp = pj()
            for ic in range(3):
                n = DIC[ic]
                for h4 in range(4):
                    nc.tensor.matmul(
                        pcp[32 * h4:32 * h4 + 32, :512],
                        VdS[:n, ic, h4],
                        Sexp[ic][:n, h4, 512 * hx:512 * hx + 512],
                        start=(ic == 0), stop=(ic == 2),
                        tile_position=(0, 32 * h4))
            nc.vector.tensor_copy(cpT[:, hx], pcp[:, :512])
            pco = pj()
            for ic in range(3):
                n = DIC[ic]
                for h4 in range(4):
                    nc.tensor.matmul(
                        pco[32 * h4:32 * h4 + 32, :512],
                        rinvb32[:n, ic, h4],
                        Sexp[ic][:n, h4, 512 * hx:512 * hx + 512],
                        start=(ic == 0), stop=(ic == 2),
                        tile_position=(0, 32 * h4))
            nc.vector.tensor_copy(csT[:, hx], pco[:, :512])
        ctxpT_sb.append(cpT)
        colsc.append(csT)

    # ---- output: drug side (out = De + pod * gd_sig) ----
    for ic in range(3):
        n = DIC[ic]
        pod = pj()
        for g in range(2):
            nc.tensor.matmul(pod[:n, :H],
                             ctxdT_sb[g][:, DOF[ic]:DOF[ic] + n],
                             wsb[OD][:, g], start=(g == 0), stop=False)
        nc.tensor.matmul(pod[:n, :H], onerr[0:1, :n], nbrod[0:1, 0],
                         start=False, stop=True)
        tmp = io.tile([128, H], F32, tag="ftmp", name="ftmp")
        nc.vector.tensor_mul(tmp[:n], pod[:n, :H], gd[:n, ic])
        ob = io.tile([128, H], F32, tag="ost", name="ost")
        nc.gpsimd.tensor_add(ob[:n], tmp[:n], De[:n, ic])
        nc.gpsimd.dma_start(out[b, DOF[ic]:DOF[ic] + n], ob[:n])

    # ---- output: protein side (out = Pe + pop * gp_sig * sigp) ----
    for jc in range(NJC):
        n = PJN[jc]
        hx, off = jc // 4, (jc % 4) * 128
        pfc = pj()
        for g in range(2):
            nc.tensor.matmul(pfc[:n, :H], colsc[g][:, hx, off:off + n],
                             wfcb[g], start=(g == 0), stop=False)
        nc.tensor.matmul(pfc[:n, :H], onerb[0:1, :n], nbb[0:1, 6],
                         start=False, stop=True)
        sigp = io.tile([128, H], BF16, tag="sigp", name="sigp")
        nc.scalar.activation(sigp[:n], pfc[:n, :H], AF.Tanh, scale=0.5)
        pop = pj()
        for g in range(2):
            nc.tensor.matmul(pop[:n, :H], ctxpT_sb[g][:, hx, off:off + n],
                             wsb[OP][:, g], start=(g == 0), stop=False)
        nc.tensor.matmul(pop[:n, :H], onerb[0:1, :n], nbb[0:1, 5],
                         start=False, stop=True)
        gg = io.tile([128, H], BF16, tag="gg", name="gg")
        nc.vector.scalar_tensor_tensor(gg[:n], sigp[:n], 0.5, gp[:n, jc],
                                       op0=ALU.mult, op1=ALU.mult)
        gg2 = io.tile([128, H], BF16, tag="gg2", name="gg2")
        nc.vector.scalar_tensor_tensor(gg2[:n], gp[:n, jc], 0.5, gg[:n],
                                       op0=ALU.mult, op1=ALU.add)
        tmp = io.tile([128, H], F32, tag="ftmp", name="ftmp")
        nc.vector.tensor_mul(tmp[:n], pop[:n, :H], gg2[:n])
        ob = io.tile([128, H], F32, tag="ost", name="ost")
        nc.gpsimd.tensor_add(ob[:n], tmp[:n], Pe[:n, jc])
        nc.gpsimd.dma_start(out[b, DL + 128 * jc:DL + 128 * jc + n], ob[:n])


def _in_maps(inputs, host):
    in_maps = []
    for c in range(NCORES):
        m = dict(drug=np.ascontiguousarray(inputs["drug"][c * BPC:(c + 1) * BPC]),
                 prot=np.ascontiguousarray(inputs["protein"][c * BPC:(c + 1) * BPC]),
                 wt=host["wt"], tb=host["tb"], nb=host["nb"],
                 wfcg=host["wfcg"], ped=host["ped"], pep=host["pep"],
                 ident=host["ident"], onesrow=host["onesrow"])
        in_maps.append(m)
    return in_maps


def kernel(**inputs):
    inputs = {k: np.asarray(v) for k, v in inputs.items()}
    host = _host_prep(inputs)
    if "nc" not in _CACHE:
        _CACHE["nc"] = _build()
    nc = _CACHE["nc"]
    res = run_bass_kernel_spmd(nc, _in_maps(inputs, host), list(range(NCORES)))
    return np.concatenate([res.results[c]["out"] for c in range(NCORES)],
                          axis=0)

